# revision 2
# baseline (speedup 1.0000x reference)
"""Multi-head attention (B=2, S=2048, D=1024, H=16) on 8 Trainium2 NeuronCores.

Sharding: data parallel over batch (4 cores per batch) x tensor parallel over
heads (4 heads per core). Each core computes Q/K/V projections for its 4 heads,
full attention for those heads, and a partial output projection; a per-batch
4-core ReduceScatter(add) produces the final output shards.

All matmuls run in float32r (fp32 storage, RNE-rounded to 11 mantissa bits on
PE ingest) which streams at full PE rate, ~2e-4 relative error.

Self-contained: only needs numpy + the concourse stack on PYTHONPATH.
"""
import functools

import numpy as np

import concourse.bass as bass
import concourse.mybir as mybir
import concourse.tile as tile
from concourse import bacc
from concourse.bass import ts, ds
from concourse.bass_utils import run_bass_kernel_spmd

TRACE = False
LAST_EXEC_NS = None

F32 = mybir.dt.float32
F32R = mybir.dt.float32r
EXP = mybir.ActivationFunctionType.Exp

B, S, D = 2, 2048, 1024
H, DH = 16, 64
NCORES = 8
GPB = 4              # cores (head-groups) per batch
HPC = H // GPB       # heads per core = 4
CH = HPC * DH        # channels per core = 256
VAUG = HPC * (DH + 1)  # V channels + one ones-column per head = 260

QB = 4               # query blocks of 512
KT = S // 128        # key token tiles = 16
XK = D // 128        # contraction tiles for projections = 8


def _build(has_bqk: bool, has_bout: bool):
    nc = bacc.Bacc("TRN2", target_bir_lowering=False, debug=False,
                   num_devices=NCORES)

    xT = nc.dram_tensor("xT", [D, S], F32, kind="ExternalInput").ap()
    wqk = nc.dram_tensor("wqk", [D, 2 * CH], F32, kind="ExternalInput").ap()
    wv = nc.dram_tensor("wv", [D, VAUG], F32, kind="ExternalInput").ap()
    wout = nc.dram_tensor("wout", [CH, D], F32, kind="ExternalInput").ap()
    bqk = nc.dram_tensor("bqk", [2 * CH], F32, kind="ExternalInput").ap()
    bv = nc.dram_tensor("bv", [VAUG], F32, kind="ExternalInput").ap()
    bout = nc.dram_tensor("bout", [D], F32, kind="ExternalInput").ap()
    y = nc.dram_tensor("y", [S // GPB, D], F32, kind="ExternalOutput").ap()

    with tile.TileContext(nc) as tc:
        with (
            tc.tile_pool(name="pw", bufs=1) as pw,           # persistent
            tc.tile_pool(name="dram", bufs=1, space="DRAM") as pd,
        ):
            # ---------------- persistent tiles ----------------
            # qkT m-slots: 0,1 = Q^T chans 0:128,128:256; 2,3 = K^T same
            qkT = pw.tile([128, 4, S], F32R)
            vt = pw.tile([128, KT, VAUG], F32R)     # [tok_in_tile, tok_tile, 4*65]
            attnT = pw.tile([128, 2, S], F32R)      # [chan, pair, token]
            wout_t = pw.tile([128, 2, D], F32R)
            ones_f = pw.tile([1, 512], F32)
            nc.vector.memset(ones_f[:], 1.0)
            ones_r = pw.tile([1, 512], F32R)
            nc.vector.tensor_copy(ones_r[:], ones_f[:])
            bv_t = pw.tile([1, VAUG], F32R)
            nc.gpsimd.dma_start(out=bv_t[:], in_=bv.unsqueeze(0))
            if has_bqk:
                bqk_t = pw.tile([1, 2 * CH], F32R)
                nc.gpsimd.dma_start(out=bqk_t[:], in_=bqk.unsqueeze(0))
            if has_bout:
                bout_t = pw.tile([1, D], F32R)
                nc.gpsimd.dma_start(out=bout_t[:], in_=bout.unsqueeze(0))

            partial = pd.tile([S, D], F32)          # pre-reduce out-proj
            rs_out = pd.tile([S // GPB, D], F32)    # reduce-scatter result

            nc.gpsimd.dma_start(
                out=wout_t[:],
                in_=wout.rearrange("(kk p) d -> p kk d", p=128))

            # ---------------- phase 1: QKV projections ----------------
            with (
                tc.tile_pool(name="px", bufs=1) as px,
                tc.tile_pool(name="psA", bufs=4, space="PSUM") as psA,
            ):
                xT_t = px.tile([128, XK, S], F32R)
                for kk in range(XK):
                    nc.gpsimd.dma_start(
                        out=xT_t[:, kk, :], in_=xT[kk * 128:(kk + 1) * 128, :])
                wqk_t = px.tile([128, XK, 2 * CH], F32R)
                nc.gpsimd.dma_start(
                    out=wqk_t[:], in_=wqk.rearrange("(kk p) c -> p kk c", p=128))
                wv_t = px.tile([128, XK, VAUG], F32R)
                nc.gpsimd.dma_start(
                    out=wv_t[:], in_=wv.rearrange("(kk p) c -> p kk c", p=128))

                # Q^T / K^T: [chan, token]
                for m in range(4):
                    for n in range(QB):
                        ps = psA.tile([128, 512], F32, tag="psqk")
                        for kk in range(XK):
                            nc.tensor.matmul(
                                ps[:],
                                lhsT=wqk_t[:, kk, ds(m * 128, 128)],
                                rhs=xT_t[:, kk, ts(n, 512)],
                                start=(kk == 0),
                                stop=(kk == XK - 1 and not has_bqk))
                        if has_bqk:
                            nc.tensor.matmul(
                                ps[:], lhsT=bqk_t[:, ds(m * 128, 128)],
                                rhs=ones_r[:], start=False, stop=True)
                        nc.vector.tensor_copy(qkT[:, m, ts(n, 512)], ps[:])

                # V (+ones columns): [token, 4*65]
                for t in range(KT):
                    psv = psA.tile([128, VAUG], F32, tag="psv")
                    for kk in range(XK):
                        nc.tensor.matmul(
                            psv[:],
                            lhsT=xT_t[:, kk, ts(t, 128)],
                            rhs=wv_t[:, kk, :],
                            start=(kk == 0), stop=False)
                    nc.tensor.matmul(
                        psv[:], lhsT=ones_r[:, 0:128], rhs=bv_t[:],
                        start=False, stop=True)
                    nc.scalar.copy(vt[:, t, :], psv[:])

            # ---------------- phase 2: attention ----------------
            with (
                tc.tile_pool(name="pe", bufs=3) as pe,
                tc.tile_pool(name="psm", bufs=1, space="PSUM") as psm,
                tc.tile_pool(name="sm", bufs=3) as sm,
            ):
                for qb in range(QB):
                    for p in range(2):
                        e_t = [pe.tile([128, KT, 512], F32R, tag="E",
                                       name=f"e_{qb}_{p}_{i}", bufs=2)
                               for i in range(2)]
                        # scores^T + exp, row-packed head pair
                        for ktg in range(KT // 2):
                            pss = [psm.tile([128, 2, 512], F32, tag="psS",
                                            name=f"pss_{qb}_{p}_{ktg}_{i}", bufs=2)
                                   for i in range(2)]
                            for j in range(2):
                                kt = ktg * 2 + j
                                for h01 in range(2):
                                    rows = ds(h01 * 64, 64)
                                    nc.tensor.matmul(
                                        pss[h01][:, j, :],
                                        lhsT=qkT[rows, 2 + p, ts(kt, 128)],
                                        rhs=qkT[rows, p, ts(qb, 512)],
                                        start=True, stop=True)
                            for h01 in range(2):
                                nc.scalar.activation(
                                    e_t[h01][:, ds(ktg * 2, 2), :],
                                    pss[h01][:], EXP, scale=0.125)
                        # U + denominator, reciprocal, broadcast, normalize
                        for h01 in range(2):
                            h = 2 * p + h01
                            psu = psm.tile([65, 512], F32, tag="psU", bufs=1)
                            for kt in range(KT):
                                nc.tensor.matmul(
                                    psu[:],
                                    lhsT=vt[:, kt, ds(h * 65, 65)],
                                    rhs=e_t[h01][:, kt, :],
                                    start=(kt == 0), stop=(kt == KT - 1))
                            rec_row = sm.tile([1, 512], F32R, tag="rec")
                            with nc.allow_low_precision(
                                    reason="softmax denom reciprocal"):
                                nc.vector.reciprocal(rec_row[:], psu[64:65, :])
                            psr = psm.tile([64, 512], F32, tag="psR", bufs=1)
                            nc.tensor.matmul(
                                psr[:], lhsT=ones_r[:, 0:64], rhs=rec_row[:],
                                start=True, stop=True)
                            recb = sm.tile([64, 512], F32, tag="recb")
                            nc.scalar.copy(recb[:], psr[:])
                            nc.vector.tensor_mul(
                                attnT[ds(h01 * 64, 64), p, ts(qb, 512)],
                                psu[0:64, :], recb[:])

                    # out-projection for this query block
                    for tt in range(4):
                        t = qb * 4 + tt
                        for n in range(2):
                            pso = psm.tile([128, 512], F32, tag="psO", bufs=2)
                            for kk in range(2):
                                nc.tensor.matmul(
                                    pso[:],
                                    lhsT=attnT[:, kk, ts(t, 128)],
                                    rhs=wout_t[:, kk, ds(n * 512, 512)],
                                    start=(kk == 0),
                                    stop=(kk == 1 and not has_bout))
                            if has_bout:
                                nc.tensor.matmul(
                                    pso[:], lhsT=ones_r[:, 0:128],
                                    rhs=bout_t[:, ds(n * 512, 512)],
                                    start=False, stop=True)
                            ob = sm.tile([128, 512], F32, tag="ob")
                            nc.vector.tensor_copy(ob[:], pso[:])
                            nc.sync.dma_start(
                                out=partial[ts(t, 128), ds(n * 512, 512)],
                                in_=ob[:])

                    # chunked reduce-scatter of this query block's rows
                    nc.gpsimd.collective_compute(
                        "ReduceScatter",
                        mybir.AluOpType.add,
                        replica_groups=[[0, 1, 2, 3], [4, 5, 6, 7]],
                        ins=[partial[ds(qb * 512, 512), :].opt()],
                        outs=[rs_out[ds(qb * 128, 128), :].opt()],
                    )
                    nc.gpsimd.dma_start(
                        out=y[ds(qb * 128, 128), :],
                        in_=rs_out[ds(qb * 128, 128), :])

    nc.compile()
    return nc


@functools.lru_cache(maxsize=None)
def _built(has_bqk: bool, has_bout: bool):
    return _build(has_bqk, has_bout)


def kernel(x, W_qkv, b_qkv, b_out, W_out=None, **_unused):
    # accept keyword order-agnostic call
    assert W_out is not None
    x = np.asarray(x, dtype=np.float32)
    W_qkv = np.asarray(W_qkv, dtype=np.float32)
    b_qkv = np.asarray(b_qkv, dtype=np.float32)
    W_out = np.asarray(W_out, dtype=np.float32)
    b_out = np.asarray(b_out, dtype=np.float32)

    has_bqk = bool(np.any(b_qkv))
    has_bout = bool(np.any(b_out))
    nc = _built(has_bqk, has_bout)

    in_maps = []
    for c in range(NCORES):
        b = c // GPB
        g = c % GPB
        ch0 = g * CH
        wq = W_qkv[:, ch0:ch0 + CH]
        wk = W_qkv[:, D + ch0:D + ch0 + CH]
        wv_raw = W_qkv[:, 2 * D + ch0:2 * D + ch0 + CH]
        wv_aug = np.zeros((D, VAUG), dtype=np.float32)
        bv_aug = np.zeros(VAUG, dtype=np.float32)
        for h in range(HPC):
            wv_aug[:, h * (DH + 1):h * (DH + 1) + DH] = \
                wv_raw[:, h * DH:(h + 1) * DH]
            bv_aug[h * (DH + 1):h * (DH + 1) + DH] = \
                b_qkv[2 * D + ch0 + h * DH:2 * D + ch0 + (h + 1) * DH]
            bv_aug[h * (DH + 1) + DH] = 1.0
        in_maps.append({
            "xT": np.ascontiguousarray(x[b].T),
            "wqk": np.ascontiguousarray(np.concatenate([wq, wk], axis=1)),
            "wv": wv_aug,
            "wout": np.ascontiguousarray(W_out[ch0:ch0 + CH, :]),
            "bqk": np.ascontiguousarray(
                np.concatenate([b_qkv[ch0:ch0 + CH],
                                b_qkv[D + ch0:D + ch0 + CH]])),
            "bv": bv_aug,
            "bout": (b_out / GPB).astype(np.float32),
        })

    global LAST_EXEC_NS
    res = run_bass_kernel_spmd(nc, in_maps, list(range(NCORES)), trace=TRACE)
    LAST_EXEC_NS = res.exec_time_ns

    out = np.empty((B, S, D), dtype=np.float32)
    for c in range(NCORES):
        b = c // GPB
        g = c % GPB
        yc = res.results[c]["y"]  # [512, D]: rows qb*128+r -> token qb*512+g*128+r
        for qb in range(QB):
            out[b, qb * 512 + g * 128: qb * 512 + (g + 1) * 128, :] = \
                yc[qb * 128:(qb + 1) * 128, :]
    return out


# revision 5
# speedup vs baseline: 1.1265x; 1.1265x over previous
"""Multi-head attention (B=2, S=2048, D=1024, H=16) on 8 Trainium2 NeuronCores.

Sharding: data parallel over batch (4 cores per batch) x tensor parallel over
heads (4 heads per core). Each core computes Q/K/V projections for its 4 heads,
full attention for those heads, and a partial output projection; a per-batch
4-core ReduceScatter(add) produces the final output shards.

All matmuls run in float32r (fp32 storage, RNE-rounded to 11 mantissa bits on
PE ingest) which streams at full PE rate, ~2e-4 relative error.
"""
import functools

import numpy as np

import concourse.bass as bass
import concourse.mybir as mybir
import concourse.tile as tile
from concourse import bacc
from concourse.bass import ts, ds
from concourse.bass_utils import run_bass_kernel_spmd

TRACE = False
LAST_EXEC_NS = None

F32 = mybir.dt.float32
F32R = mybir.dt.float32r
EXP = mybir.ActivationFunctionType.Exp

B, S, D = 2, 2048, 1024
H, DH = 16, 64
NCORES = 8
GPB = 4              # cores (head-groups) per batch
HPC = H // GPB       # heads per core = 4
CH = HPC * DH        # channels per core = 256
VAUG = HPC * (DH + 1)  # V channels + one ones-column per head = 260

QB = 4               # query blocks of 512
KT = S // 128        # key token tiles = 16
XK = D // 128        # contraction tiles for projections = 8


def _build(has_bqk: bool, has_bout: bool):
    nc = bacc.Bacc("TRN2", target_bir_lowering=False, debug=False,
                   num_devices=NCORES)

    xT = nc.dram_tensor("xT", [D, S], F32R, kind="ExternalInput").ap()
    wqk = nc.dram_tensor("wqk", [D, 2 * CH], F32R, kind="ExternalInput").ap()
    wv = nc.dram_tensor("wv", [D, VAUG], F32R, kind="ExternalInput").ap()
    wout = nc.dram_tensor("wout", [CH, D], F32R, kind="ExternalInput").ap()
    bqk = nc.dram_tensor("bqk", [2 * CH], F32R, kind="ExternalInput").ap()
    bv = nc.dram_tensor("bv", [VAUG], F32R, kind="ExternalInput").ap()
    bout = nc.dram_tensor("bout", [D], F32R, kind="ExternalInput").ap()
    y = nc.dram_tensor("y", [S // GPB, D], F32, kind="ExternalOutput").ap()

    with tile.TileContext(nc) as tc:
        with (
            tc.tile_pool(name="pw", bufs=1) as pw,           # persistent
            tc.tile_pool(name="dram", bufs=1, space="DRAM") as pd,
        ):
            # ---------------- persistent tiles ----------------
            # qkT m-slots: 0,1 = Q^T chans 0:128,128:256; 2,3 = K^T same
            qkT = pw.tile([128, 4, S], F32R)
            vt = pw.tile([128, KT, VAUG], F32R)     # [tok_in_tile, tok_tile, 4*65]
            attnT = pw.tile([128, 2, S], F32R)      # [chan, pair, token]
            wout_t = pw.tile([128, 2, D], F32R)
            ones_f = pw.tile([1, 512], F32)
            nc.vector.memset(ones_f[:], 1.0)
            ones_r = pw.tile([1, 512], F32R)
            nc.vector.tensor_copy(ones_r[:], ones_f[:])
            bv_t = pw.tile([1, VAUG], F32R)
            nc.sync.dma_start(out=bv_t[:], in_=bv.unsqueeze(0))
            if has_bqk:
                bqk_t = pw.tile([1, 2 * CH], F32R)
                nc.sync.dma_start(out=bqk_t[:], in_=bqk.unsqueeze(0))
            if has_bout:
                bout_t = pw.tile([1, D], F32R)
                nc.sync.dma_start(out=bout_t[:], in_=bout.unsqueeze(0))

            partial = pd.tile([S, D], F32)          # pre-reduce out-proj
            rs_out = pd.tile([S // GPB, D], F32)    # reduce-scatter result

            nc.sync.dma_start(
                out=wout_t[:],
                in_=wout.rearrange("(kk p) d -> p kk d", p=128))

            # ---------------- phase 1: QKV projections ----------------
            with (
                tc.tile_pool(name="px", bufs=1) as px,
                tc.tile_pool(name="psA", bufs=4, space="PSUM") as psA,
            ):
                wqk_t = px.tile([128, XK, 2 * CH], F32R)
                nc.sync.dma_start(
                    out=wqk_t[:], in_=wqk.rearrange("(kk p) c -> p kk c", p=128))
                wv_t = px.tile([128, XK, VAUG], F32R)
                nc.sync.dma_start(
                    out=wv_t[:], in_=wv.rearrange("(kk p) c -> p kk c", p=128))
                # x^T loaded in [token-chunk, k-tile] order so compute can
                # start as soon as the first token chunk lands
                xT_t = px.tile([128, XK, S], F32R)
                for n in range(QB):
                    for kk in range(XK):
                        nc.sync.dma_start(
                            out=xT_t[:, kk, ts(n, 512)],
                            in_=xT[kk * 128:(kk + 1) * 128, ts(n, 512)])

                for n in range(QB):
                    # Q^T / K^T: [chan, token]
                    for m in range(4):
                        ps = psA.tile([128, 512], F32, tag="psqk",
                                      name=f"psqk_{m}_{n}")
                        for kk in range(XK):
                            nc.tensor.matmul(
                                ps[:],
                                lhsT=wqk_t[:, kk, ds(m * 128, 128)],
                                rhs=xT_t[:, kk, ts(n, 512)],
                                start=(kk == 0),
                                stop=(kk == XK - 1 and not has_bqk))
                        if has_bqk:
                            nc.tensor.matmul(
                                ps[:], lhsT=bqk_t[:, ds(m * 128, 128)],
                                rhs=ones_r[:], start=False, stop=True)
                        nc.vector.tensor_copy(qkT[:, m, ts(n, 512)], ps[:])
                    # V (+ones columns): [token, 4*65]
                    for t in range(n * 4, n * 4 + 4):
                        psv = psA.tile([128, VAUG], F32, tag="psv",
                                       name=f"psv_{t}")
                        for kk in range(XK):
                            nc.tensor.matmul(
                                psv[:],
                                lhsT=xT_t[:, kk, ts(t, 128)],
                                rhs=wv_t[:, kk, :],
                                start=(kk == 0), stop=False)
                        nc.tensor.matmul(
                            psv[:], lhsT=ones_r[:, 0:128], rhs=bv_t[:],
                            start=False, stop=True)
                        nc.vector.tensor_copy(vt[:, t, :], psv[:])

            # ---------------- phase 2: attention ----------------
            with (
                tc.tile_pool(name="pe", bufs=3) as pe,
                tc.tile_pool(name="psm", bufs=1, space="PSUM") as psm,
                tc.tile_pool(name="sm", bufs=3) as sm,
            ):
                for qb in range(QB):
                    for p in range(2):
                        e_t = [pe.tile([128, KT, 512], F32R, tag="E",
                                       name=f"e_{qb}_{p}_{i}", bufs=3)
                               for i in range(2)]
                        psu = [psm.tile([65, 512], F32, tag="psU",
                                        name=f"psu_{qb}_{p}_{i}", bufs=2)
                               for i in range(2)]
                        # interleaved: scores^T (row-packed pair) -> exp -> U
                        for ktg in range(KT // 2):
                            pss = [psm.tile([128, 2, 512], F32, tag="psS",
                                            name=f"pss_{qb}_{p}_{ktg}_{i}",
                                            bufs=2)
                                   for i in range(2)]
                            for j in range(2):
                                kt = ktg * 2 + j
                                for h01 in range(2):
                                    rows = ds(h01 * 64, 64)
                                    nc.tensor.matmul(
                                        pss[h01][:, j, :],
                                        lhsT=qkT[rows, 2 + p, ts(kt, 128)],
                                        rhs=qkT[rows, p, ts(qb, 512)],
                                        start=True, stop=True)
                            for h01 in range(2):
                                nc.scalar.activation(
                                    e_t[h01][:, ds(ktg * 2, 2), :],
                                    pss[h01][:], EXP, scale=0.125)
                            for j in range(2):
                                kt = ktg * 2 + j
                                for h01 in range(2):
                                    h = 2 * p + h01
                                    nc.tensor.matmul(
                                        psu[h01][:],
                                        lhsT=vt[:, kt, ds(h * 65, 65)],
                                        rhs=e_t[h01][:, kt, :],
                                        start=(kt == 0), stop=(kt == KT - 1))
                        # normalize: 1/denominator, broadcast, multiply
                        for h01 in range(2):
                            den_sb = sm.tile([1, 512], F32, tag="den",
                                             name=f"den_{qb}_{p}_{h01}")
                            nc.vector.tensor_copy(den_sb[:], psu[h01][64:65, :])
                            rec_row = sm.tile([1, 512], F32, tag="rec",
                                              name=f"rec_{qb}_{p}_{h01}")
                            nc.vector.reciprocal_approx_fast(
                                rec_row[:], den_sb[:])
                            recb = sm.tile([64, 512], F32, tag="recb",
                                           name=f"recb_{qb}_{p}_{h01}")
                            nc.gpsimd.partition_broadcast(recb[:], rec_row[:])
                            nc.vector.tensor_mul(
                                attnT[ds(h01 * 64, 64), p, ts(qb, 512)],
                                psu[h01][0:64, :], recb[:])

                    # out-projection for this query block
                    for tt in range(4):
                        t = qb * 4 + tt
                        for n in range(2):
                            pso = psm.tile([128, 512], F32, tag="psO",
                                           name=f"pso_{t}_{n}", bufs=2)
                            for kk in range(2):
                                nc.tensor.matmul(
                                    pso[:],
                                    lhsT=attnT[:, kk, ts(t, 128)],
                                    rhs=wout_t[:, kk, ds(n * 512, 512)],
                                    start=(kk == 0),
                                    stop=(kk == 1 and not has_bout))
                            if has_bout:
                                nc.tensor.matmul(
                                    pso[:], lhsT=ones_r[:, 0:128],
                                    rhs=bout_t[:, ds(n * 512, 512)],
                                    start=False, stop=True)
                            ob = sm.tile([128, 512], F32, tag="ob",
                                         name=f"ob_{t}_{n}")
                            nc.vector.tensor_copy(ob[:], pso[:])
                            nc.sync.dma_start(
                                out=partial[ts(t, 128), ds(n * 512, 512)],
                                in_=ob[:])

                    # chunked reduce-scatter of this query block's rows
                    nc.gpsimd.collective_compute(
                        "ReduceScatter",
                        mybir.AluOpType.add,
                        replica_groups=[[0, 1, 2, 3], [4, 5, 6, 7]],
                        ins=[partial[ds(qb * 512, 512), :].opt()],
                        outs=[rs_out[ds(qb * 128, 128), :].opt()],
                    )
                    nc.gpsimd.dma_start(
                        out=y[ds(qb * 128, 128), :],
                        in_=rs_out[ds(qb * 128, 128), :])

    nc.compile()
    return nc


@functools.lru_cache(maxsize=None)
def _built(has_bqk: bool, has_bout: bool):
    return _build(has_bqk, has_bout)


def _round_f32r(a):
    """RNE-round fp32 to 11 mantissa bits (the f32r PE ingest format)."""
    bits = np.ascontiguousarray(a, dtype=np.float32).view(np.uint32)
    keep = np.uint32(0xFFFFF000)
    half = np.uint32(0x7FF)
    odd = (bits >> np.uint32(12)) & np.uint32(1)
    return ((bits + half + odd) & keep).view(np.float32)


def kernel(x, W_qkv, b_qkv, b_out, W_out=None, **_unused):
    assert W_out is not None
    x = np.asarray(x, dtype=np.float32)
    W_qkv = np.asarray(W_qkv, dtype=np.float32)
    b_qkv = np.asarray(b_qkv, dtype=np.float32)
    W_out = np.asarray(W_out, dtype=np.float32)
    b_out = np.asarray(b_out, dtype=np.float32)

    has_bqk = bool(np.any(b_qkv))
    has_bout = bool(np.any(b_out))
    nc = _built(has_bqk, has_bout)

    in_maps = []
    for c in range(NCORES):
        b = c // GPB
        g = c % GPB
        ch0 = g * CH
        wq = W_qkv[:, ch0:ch0 + CH]
        wk = W_qkv[:, D + ch0:D + ch0 + CH]
        wv_raw = W_qkv[:, 2 * D + ch0:2 * D + ch0 + CH]
        wv_aug = np.zeros((D, VAUG), dtype=np.float32)
        bv_aug = np.zeros(VAUG, dtype=np.float32)
        for h in range(HPC):
            wv_aug[:, h * (DH + 1):h * (DH + 1) + DH] = \
                wv_raw[:, h * DH:(h + 1) * DH]
            bv_aug[h * (DH + 1):h * (DH + 1) + DH] = \
                b_qkv[2 * D + ch0 + h * DH:2 * D + ch0 + (h + 1) * DH]
            bv_aug[h * (DH + 1) + DH] = 1.0
        in_maps.append({
            "xT": _round_f32r(x[b].T),
            "wqk": _round_f32r(np.concatenate([wq, wk], axis=1)),
            "wv": _round_f32r(wv_aug),
            "wout": _round_f32r(W_out[ch0:ch0 + CH, :]),
            "bqk": _round_f32r(
                np.concatenate([b_qkv[ch0:ch0 + CH],
                                b_qkv[D + ch0:D + ch0 + CH]])),
            "bv": _round_f32r(bv_aug),
            "bout": _round_f32r(b_out / GPB),
        })

    global LAST_EXEC_NS
    res = run_bass_kernel_spmd(nc, in_maps, list(range(NCORES)), trace=TRACE)
    LAST_EXEC_NS = res.exec_time_ns

    out = np.empty((B, S, D), dtype=np.float32)
    for c in range(NCORES):
        b = c // GPB
        g = c % GPB
        yc = res.results[c]["y"]  # [512, D]: rows qb*128+r -> token qb*512+g*128+r
        for qb in range(QB):
            out[b, qb * 512 + g * 128: qb * 512 + (g + 1) * 128, :] = \
                yc[qb * 128:(qb + 1) * 128, :]
    return out
